# revision 3
# baseline (speedup 1.0000x reference)
"""Trainium2 Bass kernel for the CNN + selective-attention bag encoder.

Reference computation (per full bag of B=1024 sentences, L=128 tokens):
  ev = wordVec[sen]; e1 = posVec1[p1]; e2 = posVec2[p2]
  c  = conv3(ev,Wv)+bv + conv3(e1,Wp1)+bp1 + conv3(e2,Wp2)+bp2   # [B,126,230]
  cnn = tanh(max_l c)                                            # [B,230]
  s   = (cnn @ attW) @ MR + MRb; w = softmax(s[:, rel])          # [B]
  att = sum_b w_b cnn_b; out = softmax(att @ MR + MRb)
  returns (out, att, cnn, w[:,None])

Sharding: data-parallel over the bag. Each of 8 cores encodes 128 sentences
(16384 tokens). The 1M-row word table is replicated into each core's DRAM
(bf16, rows padded to 256B).

Device pipeline per core:
  stage 1  32x dma_gather (one per 31250-row vocab shard, int16 local ids,
           padded with repeats) -> SBUF staging of this core's 16384 rows
           (shard-sorted dense order)
  stage 2  4x SBUF-source dma_gather(transpose=True) un-permutes staging into
           feature-major X_word[128 feats, 16384 tokens] (token k = sent*128+pos)
  pos      4x DRAM dma_gather(transpose=True) of the combined (p1,p2) pair
           table -> X_pos (features live at rows 50:60 of the padded table,
           so X = X_word + X_pos is a disjoint-support merge)
  conv     per group of 4 sentences: 2 channel-chunks x 3 window matmuls
           (K=60, N=4*126=504, bf16) accumulating in PSUM
  pool     reduce_max over positions -> CNN[115, 2*128]; tanh(x+bias); DMA out

The tiny attention tail (softmax over B, two small matvecs) runs on host
during unshard.
"""

import numpy as np
import ml_dtypes

import concourse.bass as bass
import concourse.tile as tile
from concourse import bacc, mybir
from concourse.bass_utils import run_bass_kernel_spmd

# ---- problem constants ----
WORD_TOTAL = 1_000_000
POS_TOTAL = 123
DIM = 50
DWPE = 5
C = 230
WIN = 3
REL = 53
B = 1024
L = 128

NCORES = 8
NSENT = B // NCORES          # 128 sentences/core
NTOK = NSENT * L             # 16384 tokens/core
LOUT = L - WIN + 1           # 126
NFEAT = DIM + 2 * DWPE       # 60 real features
EW = 128                     # padded row width (bf16) = 256B
CCH = C // 2                 # 115
GRP = 4                      # sentences per conv group
NGRP = NSENT // GRP          # 32
SHARD = 31250                # vocab rows per stage-1 shard
NSH = 32                     # shards (32 * 31250 = 1M)
PAD = 768                    # padded indices per shard (multinomial max >> mean 512)
NSLOT = NSH * PAD            # 24576 staging slots
TCH = 4096                   # tokens per transpose-gather chunk
PVOC = POS_TOTAL * POS_TOTAL  # 15129

F32 = mybir.dt.float32
BF16 = mybir.dt.bfloat16
I16 = mybir.dt.int16

_NC_CACHE = None


def build_nc():
    nc = bacc.Bacc("TRN2", target_bir_lowering=False, debug=False)

    idx1 = nc.dram_tensor("idx1", [128, NSLOT // 16], I16, kind="ExternalInput").ap()
    idx2 = nc.dram_tensor("idx2", [128, NTOK // 16], I16, kind="ExternalInput").ap()
    idxp = nc.dram_tensor("idxp", [128, NTOK // 16], I16, kind="ExternalInput").ap()
    wtab = nc.dram_tensor("wtab", [WORD_TOTAL, EW], BF16, kind="ExternalInput").ap()
    ptab = nc.dram_tensor("ptab", [PVOC, EW], BF16, kind="ExternalInput").ap()
    wcat = nc.dram_tensor("wcat", [64, WIN * C], BF16, kind="ExternalInput").ap()
    biaspk = nc.dram_tensor("biaspk", [CCH, 2], F32, kind="ExternalInput").ap()
    cnnt = nc.dram_tensor("cnnt", [CCH, 2 * NSENT], F32, kind="ExternalOutput").ap()

    with tile.TileContext(nc) as tc:
        with (
            tc.tile_pool(name="const", bufs=1) as const,
            tc.tile_pool(name="cpsum", bufs=6, space="PSUM") as cpsum,
        ):
            t1 = const.tile([128, NSLOT // 16], I16, tag="t1")
            nc.sync.dma_start(out=t1[:], in_=idx1)
            t2 = const.tile([128, NTOK // 16], I16, tag="t2")
            nc.sync.dma_start(out=t2[:], in_=idx2)
            tp = const.tile([128, NTOK // 16], I16, tag="tp")
            nc.sync.dma_start(out=tp[:], in_=idxp)
            w_sb = const.tile([64, WIN * C], BF16, tag="wcat")
            nc.sync.dma_start(out=w_sb[:], in_=wcat)
            bias_sb = const.tile([CCH, 2], F32, tag="bias")
            nc.sync.dma_start(out=bias_sb[:], in_=biaspk)

            # ---- stage 1: sharded word-row gather into staging ----
            staging = const.tile([128, NSLOT // 128, EW], BF16, tag="st")
            for s in range(NSH):
                nc.gpsimd.dma_gather(
                    out_ap=staging[:, s * (PAD // 128):(s + 1) * (PAD // 128), :],
                    in_ap=wtab[s * SHARD:(s + 1) * SHARD, :],
                    idxs_ap=t1[:, s * (PAD // 16):(s + 1) * (PAD // 16)],
                    num_idxs=PAD, num_idxs_reg=PAD, elem_size=EW,
                    single_packet=False,
                )

            # ---- stage 2 + pos: transpose gathers into feature-major X ----
            xw = const.tile([128, 1, NTOK], BF16, tag="xw")
            xp = const.tile([128, 1, NTOK], BF16, tag="xp")
            for c0 in range(0, NTOK, TCH):
                nc.gpsimd.dma_gather(
                    out_ap=xw[:, :, c0:c0 + TCH], in_ap=staging[:],
                    idxs_ap=t2[:, c0 // 16:(c0 + TCH) // 16],
                    num_idxs=TCH, num_idxs_reg=TCH, elem_size=EW, transpose=True,
                    single_packet=False,
                    sbuf_tokens_per_rank=128, sbuf_free_dim_per_rank=EW * 2,
                )
                nc.gpsimd.dma_gather(
                    out_ap=xp[:, :, c0:c0 + TCH], in_ap=ptab[:],
                    idxs_ap=tp[:, c0 // 16:(c0 + TCH) // 16],
                    num_idxs=TCH, num_idxs_reg=TCH, elem_size=EW, transpose=True,
                    single_packet=False,
                )

            # ---- merge (disjoint supports; exact in bf16) ----
            xv = xw[:].rearrange("p a b -> p (a b)")
            pv = xp[:].rearrange("p a b -> p (a b)")
            for c0 in range(0, NTOK, TCH):
                nc.vector.tensor_add(
                    out=xv[0:64, c0:c0 + TCH],
                    in0=xv[0:64, c0:c0 + TCH], in1=pv[0:64, c0:c0 + TCH])

            # ---- conv + max-pool ----
            x3 = xv.rearrange("p (blk t) -> p blk t", t=L)      # [128, 128, 128]
            cnn_sb = const.tile([CCH, 2 * NSENT], F32, tag="cnn")
            for g in range(NGRP):
                for ch in range(2):
                    ps = cpsum.tile([CCH, GRP * LOUT], F32, tag="cps")
                    psv = ps[:].rearrange("p (a b) -> p a b", b=LOUT)
                    for j in range(WIN):
                        nc.tensor.matmul(
                            out=psv,
                            lhsT=w_sb[0:NFEAT, j * C + ch * CCH: j * C + ch * CCH + CCH],
                            rhs=x3[0:NFEAT, GRP * g:GRP * (g + 1), j:j + LOUT],
                            start=(j == 0),
                            stop=(j == WIN - 1),
                        )
                    nc.vector.reduce_max(
                        out=cnn_sb[:, ch * NSENT + GRP * g: ch * NSENT + GRP * (g + 1)],
                        in_=psv,
                        axis=mybir.AxisListType.X,
                    )

            # ---- bias + tanh, store ----
            for ch in range(2):
                nc.scalar.activation(
                    out=cnn_sb[:, ch * NSENT:(ch + 1) * NSENT],
                    in_=cnn_sb[:, ch * NSENT:(ch + 1) * NSENT],
                    func=mybir.ActivationFunctionType.Tanh,
                    bias=bias_sb[:, ch:ch + 1],
                    scale=1.0,
                )
            nc.sync.dma_start(out=cnnt, in_=cnn_sb[:])

    nc.compile()
    return nc


def get_nc():
    global _NC_CACHE
    if _NC_CACHE is None:
        _NC_CACHE = build_nc()
    return _NC_CACHE


def _wrap16(flat):
    """int16 index layout: value k at [k%16, k//16], replicated to 128 parts."""
    n = len(flat)
    w = np.ascontiguousarray(np.asarray(flat).reshape(n // 16, 16).T).astype(np.int16)
    return np.tile(w, (8, 1))


def make_in_maps(sen, p1, p2, wordVec, posVec1, posVec2,
                 Wv, bv, Wp1, bp1, Wp2, bp2):
    sen = np.asarray(sen, dtype=np.int64)
    p1 = np.asarray(p1, dtype=np.int64)
    p2 = np.asarray(p2, dtype=np.int64)
    wordVec = np.asarray(wordVec, dtype=np.float32)
    posVec1 = np.asarray(posVec1, dtype=np.float32)
    posVec2 = np.asarray(posVec2, dtype=np.float32)

    # padded bf16 word table: cols 0:50 = wordVec
    wtab = np.zeros((WORD_TOTAL, EW), dtype=ml_dtypes.bfloat16)
    wtab[:, :DIM] = wordVec.astype(ml_dtypes.bfloat16)

    # combined position-pair table at cols 50:60
    ptab = np.zeros((PVOC, EW), dtype=ml_dtypes.bfloat16)
    ptab[:, DIM:DIM + DWPE] = np.repeat(posVec1, POS_TOTAL, axis=0).astype(ml_dtypes.bfloat16)
    ptab[:, DIM + DWPE:NFEAT] = np.tile(posVec2, (POS_TOTAL, 1)).astype(ml_dtypes.bfloat16)

    pair_full = (p1 * POS_TOTAL + p2).astype(np.int64)   # [B, L]

    # conv weights: W'[j] = concat_d(Wv[j], Wp1[j], Wp2[j]) -> [60, 230]
    Wv = np.asarray(Wv, dtype=np.float32)
    Wp1 = np.asarray(Wp1, dtype=np.float32)
    Wp2 = np.asarray(Wp2, dtype=np.float32)
    wcat = np.zeros((64, WIN * C), dtype=np.float32)
    for j in range(WIN):
        wcat[0:NFEAT, j * C:(j + 1) * C] = np.concatenate([Wv[j], Wp1[j], Wp2[j]], axis=0)
    wcat = wcat.astype(ml_dtypes.bfloat16)

    bias_total = (np.asarray(bv) + np.asarray(bp1) + np.asarray(bp2)).astype(np.float32)
    biaspk = np.ascontiguousarray(bias_total.reshape(2, CCH).T)

    in_maps = []
    for r in range(NCORES):
        sl = slice(r * NSENT, (r + 1) * NSENT)
        ids = sen[sl].reshape(-1)            # token slot k = sent*128 + pos
        pids = pair_full[sl].reshape(-1)

        shard_of = ids // SHARD
        order = np.argsort(shard_of, kind="stable")
        stage1 = np.zeros(NSLOT, np.int64)
        stage2 = np.zeros(NTOK, np.int64)
        for s in range(NSH):
            sel = order[shard_of[order] == s]
            loc = ids[sel] - s * SHARD
            n = len(sel)
            assert n <= PAD, f"shard {s} count {n} exceeds PAD={PAD}"
            stage1[s * PAD:s * PAD + n] = loc
            stage1[s * PAD + n:(s + 1) * PAD] = loc[-1] if n else 0
            stage2[sel] = s * PAD + np.arange(n)

        in_maps.append({
            "idx1": _wrap16(stage1),
            "idx2": _wrap16(stage2),
            "idxp": _wrap16(pids),
            "wtab": wtab,
            "ptab": ptab,
            "wcat": wcat,
            "biaspk": biaspk,
        })
    return in_maps


def assemble_cnn(results):
    parts = []
    for r in range(NCORES):
        x = np.asarray(results[r]["cnnt"], dtype=np.float32).reshape(CCH, 2, NSENT)
        parts.append(np.transpose(x, (2, 1, 0)).reshape(NSENT, C))
    return np.concatenate(parts, axis=0)


def host_epilogue(cnn, attW, MR, MRb, rel):
    attW = np.asarray(attW, dtype=np.float64)
    MR = np.asarray(MR, dtype=np.float64)
    MRb = np.asarray(MRb, dtype=np.float64)
    rel = int(np.asarray(rel))
    cnn64 = cnn.astype(np.float64)

    s = cnn64 @ (attW @ MR[:, rel]) + MRb[rel]
    s = s - s.max()
    w = np.exp(s)
    w = w / w.sum()
    att = w @ cnn64
    logits = att @ MR + MRb
    logits = logits - logits.max()
    out = np.exp(logits)
    out = out / out.sum()
    return (
        out[None, :].astype(np.float32),
        att[None, :].astype(np.float32),
        cnn.astype(np.float32),
        w[:, None].astype(np.float32),
    )


def run_device(in_maps, **kwargs):
    nc = get_nc()
    return run_bass_kernel_spmd(
        nc, in_maps, core_ids=list(range(NCORES)), **kwargs
    )


def kernel(sen, p1, p2, wordVec, posVec1, posVec2, Wv, bv, Wp1, bp1,
           Wp2, bp2, attW, MR, MRb, rel):
    in_maps = make_in_maps(sen, p1, p2, wordVec, posVec1, posVec2,
                           Wv, bv, Wp1, bp1, Wp2, bp2)
    res = run_device(in_maps)
    cnn = assemble_cnn(res.results)
    return host_epilogue(cnn, attW, MR, MRb, rel)
